# revision 15
# baseline (speedup 1.0000x reference)
"""Asymmetric correlation kernel v4 — g-outer column-tile Gram-band, bf16.

Differences vs v3:
  - Loop order: y-tile g outer (3), then x-quad q (40): each psum tile holds
    4 columns x 32 rows = 128 partitions (full-width evacuations).
  - Row-chunked input loads (full-width rows -> fat descriptors) unlock
    compute after the first chunk.
  - bandt is full-size [128, 160*81] (no ring); transposes + assembly +
    output run overlapped with the g=2 pass.
"""

from contextlib import ExitStack

import numpy as np

import concourse.bass as bass
import concourse.mybir as mybir
from concourse.bass_utils import run_bass_kernel_spmd

F32 = mybir.dt.float32
BF16 = mybir.dt.bfloat16

C = 256
H = 96
W = 160
PAD = 4
ND = 81
YT = 32
NG = H // YT            # 3
WU = YT + 2 * PAD       # 40
WV = 9
NBHD = WU * WV          # 360
X2R = H + 2 * PAD       # 104
X2C = W + 2 * PAD       # 168
XQ = 4                  # columns per psum tile
NQ = W // XQ            # 40 x-quads
SB_Q = 4                # x-quads per scratch-write batch (16 columns)
NSB = NQ // SB_Q        # 10 scratch batches per g
XB = 8                  # columns per band-gather batch
NBB = W // XB           # 20 band batches per g
LAG = 28                # transpose lag (columns) behind g=2 band gathers
GRAMBUFS = 8            # gram ring (x-quads)
KS = 4
KB = 4


def build():
    nc = bass.Bass("TRN2", target_bir_lowering=False, debug=False)

    x1 = nc.dram_tensor("x1", [C, H, W], F32, kind="ExternalInput")
    x2 = nc.dram_tensor("x2", [C, H, W], F32, kind="ExternalInput")
    out = nc.dram_tensor("out", [ND, H, W], F32, kind="ExternalOutput")
    scratch = nc.dram_tensor("scratch", [96, W, NBHD], BF16)
    SROW = W * NBHD  # 57600

    with ExitStack() as ctx:
        ent = ctx.enter_context
        x1s = ent(nc.sbuf_tensor("x1s", [128, 2, H, W], BF16))
        x2s = ent(nc.sbuf_tensor("x2s", [128, 2, X2R, X2C], BF16))
        gram = ent(nc.sbuf_tensor("gram", [128, GRAMBUFS, NBHD], BF16))
        bandt = ent(nc.sbuf_tensor("bandt", [128, W, ND], BF16))
        asm = ent(nc.sbuf_tensor("asm", [128, H, W], BF16))
        ident = ent(nc.sbuf_tensor("ident", [128, 128], BF16))

        pg = [ent(nc.psum_tensor(f"pg{i}", [128, 512], F32)) for i in range(3)]
        pt = [ent(nc.psum_tensor(f"pt{i}", [ND, 96], BF16)) for i in range(3)]

        s_init = ent(nc.semaphore("s_init"))
        s_vinit = ent(nc.semaphore("s_vinit"))
        sx = [ent(nc.semaphore(f"sx{i}")) for i in range(NG)]
        ss = [ent(nc.semaphore(f"ss{i}")) for i in range(KS)]
        sb = [ent(nc.semaphore(f"sb{i}")) for i in range(KB)]
        s_out = ent(nc.semaphore("s_out"))
        s_mm = ent(nc.semaphore("s_mm"))    # per (g, q): count g*NQ + q + 1
        s_tp = ent(nc.semaphore("s_tp"))
        s_ev1 = ent(nc.semaphore("s_ev1"))  # per (g, q)
        s_ev2 = ent(nc.semaphore("s_ev2"))

        def transpose_band(pe, k):
            B = 2 * NBB + k // XB  # band batch index (g=2 pass)
            pe.wait_ge(sb[B % KB], 16 * (B // KB + 1))
            if k >= 3:
                pe.wait_ge(s_ev2, k - 2)
            pe_in = bass.AP(
                tensor=bandt,
                offset=k * ND,
                ap=[[W * ND, 96], [1, ND]],
            )
            nc.tensor.transpose(
                pt[k % 3].ap(), pe_in, ident.ap()[0:96, 0:96]
            ).then_inc(s_tp, 1)

        def evac2(vec, k):
            vec.wait_ge(s_tp, k + 1)
            dst = bass.AP(tensor=asm, offset=k, ap=[[H * W, ND], [W, H]])
            vec.tensor_copy(dst, pt[k % 3].ap()).then_inc(s_ev2, 1)

        with nc.Block() as block:

            @block.gpsimd
            def _(gp):
                # row-chunked loads, fat descriptors (full-width rows)
                # chunk g: x1 rows [32g, 32g+32), x2 rows: g0 [0,36), g1
                # [36, 68), g2 [68, 96)  (into padded slots +PAD)
                x2rows = [(0, 36), (36, 68), (68, 96)]
                gp.wait_ge(s_vinit, 1)
                gp.affine_select(
                    out=ident.ap(),
                    in_=ident.ap(),
                    compare_op=mybir.AluOpType.not_equal,
                    fill=1.0,
                    base=0,
                    pattern=[[-1, 128]],
                    channel_multiplier=1,
                ).then_inc(s_init, 1)
                for g in range(NG):
                    if g > 0:
                        gp.wait_ge(sx[g - 1], 64)
                    r0, r1 = x2rows[g]
                    for h in range(2):
                        in1 = bass.AP(
                            tensor=x1,
                            offset=128 * h * H * W + YT * g * W,
                            ap=[[H * W, 128], [1, YT * W]],
                        )
                        gp.dma_start(
                            out=x1s.ap()[:, h, YT * g : YT * g + YT, :],
                            in_=in1,
                        ).then_inc(sx[g], 16)
                        in2 = bass.AP(
                            tensor=x2,
                            offset=128 * h * H * W + r0 * W,
                            ap=[[H * W, 128], [W, r1 - r0], [1, W]],
                        )
                        gp.dma_start(
                            out=x2s.ap()[
                                :, h, PAD + r0 : PAD + r1, PAD : PAD + W
                            ],
                            in_=in2,
                        ).then_inc(sx[g], 16)
                gp.wait_ge(s_ev2, W)
                gp.dma_start(out=out.ap(), in_=asm.ap()[0:ND, :, :]).then_inc(
                    s_out, 16
                )

            @block.vector
            def _(vec):
                for h in range(2):
                    vec.memset(x2s.ap()[:, h, :, 0:PAD], 0.0)
                    vec.memset(x2s.ap()[:, h, :, X2C - PAD :], 0.0)
                    vec.memset(x2s.ap()[:, h, 0:PAD, PAD : PAD + W], 0.0)
                    vec.memset(x2s.ap()[:, h, X2R - PAD :, PAD : PAD + W], 0.0)
                vec.memset(ident.ap(), 0.0).then_inc(s_vinit, 1)

                for g in range(NG):
                    for q in range(NQ):
                        t = g * NQ + q
                        if g == 2:
                            x_hi = XQ * q + XQ - 1  # computed through col x_hi
                            k = x_hi - LAG
                            for kk in range(max(0, k - XQ + 1), max(0, k + 1)):
                                evac2(vec, kk)
                        vec.wait_ge(s_mm, t + 1)
                        if t >= GRAMBUFS:
                            tw = t - GRAMBUFS
                            vec.wait_ge(ss[tw % KS], 16 * (tw // KS + 1))
                        vec.tensor_scalar_mul(
                            gram.ap()[:, t % GRAMBUFS, :],
                            pg[t % 3].ap()[:, 0:NBHD],
                            1.0 / C,
                        ).then_inc(s_ev1, 1)
                for k in range(max(0, W - LAG), W):
                    evac2(vec, k)

            @block.sync
            def _(sp):
                for g in range(NG):
                    for q in range(NQ):
                        t = g * NQ + q
                        sp.wait_ge(s_ev1, t + 1)
                        if t >= KS:
                            sp.wait_ge(ss[t % KS], 16 * (t // KS))
                        sp.dma_start(
                            out=bass.AP(
                                tensor=scratch,
                                offset=YT * g * SROW + q * XQ * NBHD,
                                ap=[[NBHD, XQ], [SROW, YT], [1, NBHD]],
                            ),
                            in_=bass.AP(
                                tensor=gram,
                                offset=(t % GRAMBUFS) * NBHD,
                                ap=[[GRAMBUFS * NBHD, 128], [1, NBHD]],
                            ),
                        ).then_inc(ss[t % KS], 16)

            @block.scalar
            def _(act):
                for g in range(NG):
                    for B in range(NBB):
                        gb = g * NBB + B
                        tq0 = g * NQ + 2 * B
                        for tq in (tq0, tq0 + 1):
                            act.wait_ge(ss[tq % KS], 16 * (tq // KS + 1))
                        if gb >= KB:
                            act.wait_ge(sb[gb % KB], 16 * (gb // KB))
                        in_ap = bass.AP(
                            tensor=scratch,
                            offset=YT * g * SROW + B * XB * NBHD,
                            ap=[[SROW + WV, YT], [NBHD, XB], [1, ND]],
                        )
                        out_ap = bass.AP(
                            tensor=bandt,
                            offset=YT * g * (W * ND) + B * XB * ND,
                            ap=[[W * ND, YT], [ND, XB], [1, ND]],
                        )
                        act.dma_start(out=out_ap, in_=in_ap).then_inc(
                            sb[gb % KB], 16
                        )

            @block.tensor
            def _(pe):
                pe.wait_ge(s_init, 1)
                for g in range(NG):
                    pe.wait_ge(sx[g], 64)
                    for q in range(NQ):
                        t = g * NQ + q
                        if t >= 3:
                            pe.wait_ge(s_ev1, t - 2)
                        last = None
                        for xj in range(XQ):
                            x = XQ * q + xj
                            for h in range(2):
                                lhsT = bass.AP(
                                    tensor=x1s,
                                    offset=h * H * W + YT * g * W + x,
                                    ap=[[2 * H * W, 128], [W, YT]],
                                )
                                rhs = bass.AP(
                                    tensor=x2s,
                                    offset=h * X2R * X2C + YT * g * X2C + x,
                                    ap=[
                                        [2 * X2R * X2C, 128],
                                        [X2C, WU],
                                        [1, WV],
                                    ],
                                )
                                last = nc.tensor.matmul(
                                    pg[t % 3].ap()[
                                        YT * xj : YT * xj + YT, 0:NBHD
                                    ],
                                    lhsT,
                                    rhs,
                                    start=(h == 0),
                                    stop=(h == 1),
                                    tile_position=(0, YT * xj),
                                )
                        last.then_inc(s_mm, 1)
                        if g == 2:
                            x_hi = XQ * q + XQ - 1
                            k = x_hi - LAG
                            for kk in range(max(0, k - XQ + 1), max(0, k + 1)):
                                transpose_band(pe, kk)
                for k in range(max(0, W - LAG), W):
                    transpose_band(pe, k)

    return nc


def kernel(x1, x2, trace=False):
    n = x1.shape[0]
    nc = build()
    in_maps = [
        {
            "x1": np.ascontiguousarray(x1[i], dtype=np.float32),
            "x2": np.ascontiguousarray(x2[i], dtype=np.float32),
        }
        for i in range(n)
    ]
    res = run_bass_kernel_spmd(nc, in_maps, list(range(n)), trace=trace)
    outv = np.stack([r["out"] for r in res.results], axis=0)
    if trace:
        kernel.last_exec_time_ns = res.exec_time_ns
        kernel.last_trace = res.instructions_and_trace
    return outv


# revision 17
# speedup vs baseline: 1.0792x; 1.0792x over previous
"""Asymmetric correlation kernel v4 — g-outer column-tile Gram-band, bf16.

Differences vs v3:
  - Loop order: y-tile g outer (3), then x-quad q (40): each psum tile holds
    4 columns x 32 rows = 128 partitions (full-width evacuations).
  - Row-chunked input loads (full-width rows -> fat descriptors) unlock
    compute after the first chunk.
  - bandt is full-size [128, 160*81] (no ring); transposes + assembly +
    output run overlapped with the g=2 pass.
"""

from contextlib import ExitStack

import numpy as np

import concourse.bass as bass
import concourse.mybir as mybir
from concourse.bass_utils import run_bass_kernel_spmd

F32 = mybir.dt.float32
BF16 = mybir.dt.bfloat16

C = 256
H = 96
W = 160
PAD = 4
ND = 81
YT = 32
NG = H // YT            # 3
WU = YT + 2 * PAD       # 40
WV = 9
NBHD = WU * WV          # 360
X2R = H + 2 * PAD       # 104
X2C = W + 2 * PAD       # 168
XQ = 4                  # columns per psum tile
NQ = W // XQ            # 40 x-quads
SB_Q = 4                # x-quads per scratch-write batch (16 columns)
NSB = NQ // SB_Q        # 10 scratch batches per g
XB = 8                  # columns per band-gather batch
NBB = W // XB           # 20 band batches per g
LAG = 28                # transpose lag (columns) behind g=2 band gathers
GRAMBUFS = 8            # gram ring (x-quads)
KS = 4
KB = 4


def build():
    nc = bass.Bass("TRN2", target_bir_lowering=False, debug=False)

    x1 = nc.dram_tensor("x1", [C, H, W], F32, kind="ExternalInput")
    x2 = nc.dram_tensor("x2", [C, H, W], F32, kind="ExternalInput")
    out = nc.dram_tensor("out", [ND, H, W], F32, kind="ExternalOutput")
    scratch = nc.dram_tensor("scratch", [96, W, NBHD], BF16)
    SROW = W * NBHD  # 57600

    with ExitStack() as ctx:
        ent = ctx.enter_context
        x1s = ent(nc.sbuf_tensor("x1s", [128, 2, H, W], BF16))
        x2s = ent(nc.sbuf_tensor("x2s", [128, 2, X2R, X2C], BF16))
        gram = ent(nc.sbuf_tensor("gram", [128, GRAMBUFS, NBHD], BF16))
        bandt = ent(nc.sbuf_tensor("bandt", [128, W, ND], BF16))
        asm = ent(nc.sbuf_tensor("asm", [128, H, W], BF16))
        ident = ent(nc.sbuf_tensor("ident", [128, 128], BF16))

        pg = [ent(nc.psum_tensor(f"pg{i}", [128, 512], F32)) for i in range(4)]
        pt = [ent(nc.psum_tensor(f"pt{i}", [ND, 96], BF16)) for i in range(4)]

        s_init = ent(nc.semaphore("s_init"))
        s_vinit = ent(nc.semaphore("s_vinit"))
        sx = [ent(nc.semaphore(f"sx{i}")) for i in range(NG)]
        ss = [ent(nc.semaphore(f"ss{i}")) for i in range(KS)]
        sb = [ent(nc.semaphore(f"sb{i}")) for i in range(KB)]
        s_out = ent(nc.semaphore("s_out"))
        s_mm = ent(nc.semaphore("s_mm"))    # per (g, q): count g*NQ + q + 1
        s_tp = ent(nc.semaphore("s_tp"))
        s_ev1 = ent(nc.semaphore("s_ev1"))  # per (g, q)
        s_ev2 = ent(nc.semaphore("s_ev2"))

        def transpose_band(pe, k):
            B = 2 * NBB + k // XB  # band batch index (g=2 pass)
            pe.wait_ge(sb[B % KB], 16 * (B // KB + 1))
            if k >= 4:
                pe.wait_ge(s_ev2, k - 3)
            pe_in = bass.AP(
                tensor=bandt,
                offset=k * ND,
                ap=[[W * ND, 96], [1, ND]],
            )
            nc.tensor.transpose(
                pt[k % 4].ap(), pe_in, ident.ap()[0:96, 0:96]
            ).then_inc(s_tp, 1)

        def evac2(vec, k):
            vec.wait_ge(s_tp, k + 1)
            dst = bass.AP(tensor=asm, offset=k, ap=[[H * W, ND], [W, H]])
            vec.tensor_copy(dst, pt[k % 4].ap()).then_inc(s_ev2, 1)

        with nc.Block() as block:

            @block.gpsimd
            def _(gp):
                # row-chunked loads, fat descriptors (full-width rows)
                # chunk g: x1 rows [32g, 32g+32), x2 rows: g0 [0,36), g1
                # [36, 68), g2 [68, 96)  (into padded slots +PAD)
                x2rows = [(0, 36), (36, 68), (68, 96)]
                gp.wait_ge(s_vinit, 1)
                gp.affine_select(
                    out=ident.ap(),
                    in_=ident.ap(),
                    compare_op=mybir.AluOpType.not_equal,
                    fill=1.0,
                    base=0,
                    pattern=[[-1, 128]],
                    channel_multiplier=1,
                ).then_inc(s_init, 1)
                for g in range(NG):
                    if g > 0:
                        gp.wait_ge(sx[g - 1], 64)
                    r0, r1 = x2rows[g]
                    for h in range(2):
                        in1 = bass.AP(
                            tensor=x1,
                            offset=128 * h * H * W + YT * g * W,
                            ap=[[H * W, 128], [1, YT * W]],
                        )
                        gp.dma_start(
                            out=x1s.ap()[:, h, YT * g : YT * g + YT, :],
                            in_=in1,
                        ).then_inc(sx[g], 16)
                        in2 = bass.AP(
                            tensor=x2,
                            offset=128 * h * H * W + r0 * W,
                            ap=[[H * W, 128], [W, r1 - r0], [1, W]],
                        )
                        gp.dma_start(
                            out=x2s.ap()[
                                :, h, PAD + r0 : PAD + r1, PAD : PAD + W
                            ],
                            in_=in2,
                        ).then_inc(sx[g], 16)
                gp.wait_ge(s_ev2, W)
                gp.dma_start(out=out.ap(), in_=asm.ap()[0:ND, :, :]).then_inc(
                    s_out, 16
                )

            @block.vector
            def _(vec):
                for h in range(2):
                    vec.memset(x2s.ap()[:, h, :, 0:PAD], 0.0)
                    vec.memset(x2s.ap()[:, h, :, X2C - PAD :], 0.0)
                    vec.memset(x2s.ap()[:, h, 0:PAD, PAD : PAD + W], 0.0)
                    vec.memset(x2s.ap()[:, h, X2R - PAD :, PAD : PAD + W], 0.0)
                vec.memset(ident.ap(), 0.0).then_inc(s_vinit, 1)

                for g in range(NG):
                    for q in range(NQ):
                        t = g * NQ + q
                        if g == 2:
                            x_hi = XQ * q + XQ - 1  # computed through col x_hi
                            k = x_hi - LAG
                            for kk in range(max(0, k - XQ + 1), max(0, k + 1)):
                                evac2(vec, kk)
                        vec.wait_ge(s_mm, t + 1)
                        if t >= GRAMBUFS:
                            tw = t - GRAMBUFS
                            vec.wait_ge(ss[tw % KS], 16 * (tw // KS + 1))
                        vec.tensor_scalar_mul(
                            gram.ap()[:, t % GRAMBUFS, :],
                            pg[t % 4].ap()[:, 0:NBHD],
                            1.0 / C,
                        ).then_inc(s_ev1, 1)
                for k in range(max(0, W - LAG), W):
                    evac2(vec, k)

            @block.sync
            def _(sp):
                for g in range(NG):
                    for q in range(NQ):
                        t = g * NQ + q
                        sp.wait_ge(s_ev1, t + 1)
                        if t >= KS:
                            sp.wait_ge(ss[t % KS], 16 * (t // KS))
                        sp.dma_start(
                            out=bass.AP(
                                tensor=scratch,
                                offset=YT * g * SROW + q * XQ * NBHD,
                                ap=[[NBHD, XQ], [SROW, YT], [1, NBHD]],
                            ),
                            in_=bass.AP(
                                tensor=gram,
                                offset=(t % GRAMBUFS) * NBHD,
                                ap=[[GRAMBUFS * NBHD, 128], [1, NBHD]],
                            ),
                        ).then_inc(ss[t % KS], 16)

            @block.scalar
            def _(act):
                for g in range(NG):
                    for B in range(NBB):
                        gb = g * NBB + B
                        tq0 = g * NQ + 2 * B
                        for tq in (tq0, tq0 + 1):
                            act.wait_ge(ss[tq % KS], 16 * (tq // KS + 1))
                        if gb >= KB:
                            act.wait_ge(sb[gb % KB], 16 * (gb // KB))
                        in_ap = bass.AP(
                            tensor=scratch,
                            offset=YT * g * SROW + B * XB * NBHD,
                            ap=[[SROW + WV, YT], [NBHD, XB], [1, ND]],
                        )
                        out_ap = bass.AP(
                            tensor=bandt,
                            offset=YT * g * (W * ND) + B * XB * ND,
                            ap=[[W * ND, YT], [ND, XB], [1, ND]],
                        )
                        act.dma_start(out=out_ap, in_=in_ap).then_inc(
                            sb[gb % KB], 16
                        )

            @block.tensor
            def _(pe):
                pe.wait_ge(s_init, 1)
                for g in range(NG):
                    pe.wait_ge(sx[g], 64)
                    for q in range(NQ):
                        t = g * NQ + q
                        if t >= 4:
                            pe.wait_ge(s_ev1, t - 3)
                        last = None
                        for xj in range(XQ):
                            x = XQ * q + xj
                            for h in range(2):
                                lhsT = bass.AP(
                                    tensor=x1s,
                                    offset=h * H * W + YT * g * W + x,
                                    ap=[[2 * H * W, 128], [W, YT]],
                                )
                                rhs = bass.AP(
                                    tensor=x2s,
                                    offset=h * X2R * X2C + YT * g * X2C + x,
                                    ap=[
                                        [2 * X2R * X2C, 128],
                                        [X2C, WU],
                                        [1, WV],
                                    ],
                                )
                                last = nc.tensor.matmul(
                                    pg[t % 4].ap()[
                                        YT * xj : YT * xj + YT, 0:NBHD
                                    ],
                                    lhsT,
                                    rhs,
                                    start=(h == 0),
                                    stop=(h == 1),
                                    tile_position=(0, YT * xj),
                                )
                        last.then_inc(s_mm, 1)
                        if g == 2:
                            x_hi = XQ * q + XQ - 1
                            k = x_hi - LAG
                            for kk in range(max(0, k - XQ + 1), max(0, k + 1)):
                                transpose_band(pe, kk)
                for k in range(max(0, W - LAG), W):
                    transpose_band(pe, k)

    return nc


def kernel(x1, x2, trace=False):
    n = x1.shape[0]
    nc = build()
    in_maps = [
        {
            "x1": np.ascontiguousarray(x1[i], dtype=np.float32),
            "x2": np.ascontiguousarray(x2[i], dtype=np.float32),
        }
        for i in range(n)
    ]
    res = run_bass_kernel_spmd(nc, in_maps, list(range(n)), trace=trace)
    outv = np.stack([r["out"] for r in res.results], axis=0)
    if trace:
        kernel.last_exec_time_ns = res.exec_time_ns
        kernel.last_trace = res.instructions_and_trace
    return outv


# revision 19
# speedup vs baseline: 1.1113x; 1.0298x over previous
"""Asymmetric correlation kernel v4 — g-outer column-tile Gram-band, bf16.

Differences vs v3:
  - Loop order: y-tile g outer (3), then x-quad q (40): each psum tile holds
    4 columns x 32 rows = 128 partitions (full-width evacuations).
  - Row-chunked input loads (full-width rows -> fat descriptors) unlock
    compute after the first chunk.
  - bandt is full-size [128, 160*81] (no ring); transposes + assembly +
    output run overlapped with the g=2 pass.
"""

from contextlib import ExitStack

import numpy as np

import concourse.bass as bass
import concourse.mybir as mybir
from concourse.bass_utils import run_bass_kernel_spmd

F32 = mybir.dt.float32
BF16 = mybir.dt.bfloat16

C = 256
H = 96
W = 160
PAD = 4
ND = 81
YT = 32
NG = H // YT            # 3
WU = YT + 2 * PAD       # 40
WV = 9
NBHD = WU * WV          # 360
X2R = H + 2 * PAD       # 104
X2C = W + 2 * PAD       # 168
XQ = 4                  # columns per psum tile
NQ = W // XQ            # 40 x-quads
SB_Q = 4                # x-quads per scratch-write batch (16 columns)
NSB = NQ // SB_Q        # 10 scratch batches per g
XB = 8                  # columns per band-gather batch
NBB = W // XB           # 20 band batches per g
LAG = 28                # transpose lag (columns) behind g=2 band gathers
GRAMBUFS = 12           # gram ring (x-quads)
KS = 6
KB = 4


def build():
    nc = bass.Bass("TRN2", target_bir_lowering=False, debug=False)

    x1 = nc.dram_tensor("x1", [C, H, W], F32, kind="ExternalInput")
    x2 = nc.dram_tensor("x2", [C, H, W], F32, kind="ExternalInput")
    out = nc.dram_tensor("out", [ND, H, W], F32, kind="ExternalOutput")
    scratch = nc.dram_tensor("scratch", [96, W, NBHD], BF16)
    SROW = W * NBHD  # 57600

    with ExitStack() as ctx:
        ent = ctx.enter_context
        x1s = ent(nc.sbuf_tensor("x1s", [128, 2, H, W], BF16))
        x2s = ent(nc.sbuf_tensor("x2s", [128, 2, X2R, X2C], BF16))
        gram = ent(nc.sbuf_tensor("gram", [128, GRAMBUFS, NBHD], BF16))
        bandt = ent(nc.sbuf_tensor("bandt", [128, W, ND], BF16))
        asm = ent(nc.sbuf_tensor("asm", [128, H, W], BF16))
        ident = ent(nc.sbuf_tensor("ident", [128, 128], BF16))

        pg = [ent(nc.psum_tensor(f"pg{i}", [128, 512], F32)) for i in range(4)]
        pt = [ent(nc.psum_tensor(f"pt{i}", [ND, 96], BF16)) for i in range(4)]

        s_init = ent(nc.semaphore("s_init"))
        s_vinit = ent(nc.semaphore("s_vinit"))
        sx = [ent(nc.semaphore(f"sx{i}")) for i in range(NG)]
        ss = [ent(nc.semaphore(f"ss{i}")) for i in range(KS)]
        sb = [ent(nc.semaphore(f"sb{i}")) for i in range(KB)]
        s_out = ent(nc.semaphore("s_out"))
        s_mm = ent(nc.semaphore("s_mm"))    # per (g, q): count g*NQ + q + 1
        s_tp = ent(nc.semaphore("s_tp"))
        s_ev1 = ent(nc.semaphore("s_ev1"))  # per (g, q)
        s_ev2 = ent(nc.semaphore("s_ev2"))

        def transpose_band(pe, k):
            B = 2 * NBB + k // XB  # band batch index (g=2 pass)
            pe.wait_ge(sb[B % KB], 16 * (B // KB + 1))
            if k >= 4:
                pe.wait_ge(s_ev2, k - 3)
            pe_in = bass.AP(
                tensor=bandt,
                offset=k * ND,
                ap=[[W * ND, 96], [1, ND]],
            )
            nc.tensor.transpose(
                pt[k % 4].ap(), pe_in, ident.ap()[0:96, 0:96]
            ).then_inc(s_tp, 1)

        def evac2(vec, k):
            vec.wait_ge(s_tp, k + 1)
            dst = bass.AP(tensor=asm, offset=k, ap=[[H * W, ND], [W, H]])
            vec.tensor_copy(dst, pt[k % 4].ap()).then_inc(s_ev2, 1)

        with nc.Block() as block:

            @block.gpsimd
            def _(gp):
                # row-chunked loads, fat descriptors (full-width rows)
                # chunk g: x1 rows [32g, 32g+32), x2 rows: g0 [0,36), g1
                # [36, 68), g2 [68, 96)  (into padded slots +PAD)
                x2rows = [(0, 36), (36, 68), (68, 96)]
                gp.wait_ge(s_vinit, 1)
                gp.affine_select(
                    out=ident.ap(),
                    in_=ident.ap(),
                    compare_op=mybir.AluOpType.not_equal,
                    fill=1.0,
                    base=0,
                    pattern=[[-1, 128]],
                    channel_multiplier=1,
                ).then_inc(s_init, 1)
                for g in range(NG):
                    if g > 0:
                        gp.wait_ge(sx[g - 1], 64)
                    r0, r1 = x2rows[g]
                    for h in range(2):
                        in1 = bass.AP(
                            tensor=x1,
                            offset=128 * h * H * W + YT * g * W,
                            ap=[[H * W, 128], [1, YT * W]],
                        )
                        gp.dma_start(
                            out=x1s.ap()[:, h, YT * g : YT * g + YT, :],
                            in_=in1,
                        ).then_inc(sx[g], 16)
                        in2 = bass.AP(
                            tensor=x2,
                            offset=128 * h * H * W + r0 * W,
                            ap=[[H * W, 128], [W, r1 - r0], [1, W]],
                        )
                        gp.dma_start(
                            out=x2s.ap()[
                                :, h, PAD + r0 : PAD + r1, PAD : PAD + W
                            ],
                            in_=in2,
                        ).then_inc(sx[g], 16)
                gp.wait_ge(s_ev2, W)
                gp.dma_start(out=out.ap(), in_=asm.ap()[0:ND, :, :]).then_inc(
                    s_out, 16
                )

            @block.vector
            def _(vec):
                for h in range(2):
                    vec.memset(x2s.ap()[:, h, :, 0:PAD], 0.0)
                    vec.memset(x2s.ap()[:, h, :, X2C - PAD :], 0.0)
                    vec.memset(x2s.ap()[:, h, 0:PAD, PAD : PAD + W], 0.0)
                    vec.memset(x2s.ap()[:, h, X2R - PAD :, PAD : PAD + W], 0.0)
                vec.memset(ident.ap(), 0.0).then_inc(s_vinit, 1)

                for g in range(NG):
                    for q in range(NQ):
                        t = g * NQ + q
                        if g == 2:
                            x_hi = XQ * q + XQ - 1  # computed through col x_hi
                            k = x_hi - LAG
                            for kk in range(max(0, k - XQ + 1), max(0, k + 1)):
                                evac2(vec, kk)
                        vec.wait_ge(s_mm, t + 1)
                        if t >= GRAMBUFS:
                            tw = t - GRAMBUFS
                            vec.wait_ge(ss[tw % KS], 16 * (tw // KS + 1))
                        vec.tensor_scalar_mul(
                            gram.ap()[:, t % GRAMBUFS, :],
                            pg[t % 4].ap()[:, 0:NBHD],
                            1.0 / C,
                        ).then_inc(s_ev1, 1)
                for k in range(max(0, W - LAG), W):
                    evac2(vec, k)

            @block.sync
            def _(sp):
                for g in range(NG):
                    for q in range(NQ):
                        t = g * NQ + q
                        sp.wait_ge(s_ev1, t + 1)
                        if t >= KS:
                            sp.wait_ge(ss[t % KS], 16 * (t // KS))
                        sp.dma_start(
                            out=bass.AP(
                                tensor=scratch,
                                offset=YT * g * SROW + q * XQ * NBHD,
                                ap=[[NBHD, XQ], [SROW, YT], [1, NBHD]],
                            ),
                            in_=bass.AP(
                                tensor=gram,
                                offset=(t % GRAMBUFS) * NBHD,
                                ap=[[GRAMBUFS * NBHD, 128], [1, NBHD]],
                            ),
                        ).then_inc(ss[t % KS], 16)

            @block.scalar
            def _(act):
                for g in range(NG):
                    for B in range(NBB):
                        gb = g * NBB + B
                        tq0 = g * NQ + 2 * B
                        for tq in (tq0, tq0 + 1):
                            act.wait_ge(ss[tq % KS], 16 * (tq // KS + 1))
                        if gb >= KB:
                            act.wait_ge(sb[gb % KB], 16 * (gb // KB))
                        in_ap = bass.AP(
                            tensor=scratch,
                            offset=YT * g * SROW + B * XB * NBHD,
                            ap=[[SROW + WV, YT], [NBHD, XB], [1, ND]],
                        )
                        out_ap = bass.AP(
                            tensor=bandt,
                            offset=YT * g * (W * ND) + B * XB * ND,
                            ap=[[W * ND, YT], [ND, XB], [1, ND]],
                        )
                        act.dma_start(out=out_ap, in_=in_ap).then_inc(
                            sb[gb % KB], 16
                        )

            @block.tensor
            def _(pe):
                pe.wait_ge(s_init, 1)
                for g in range(NG):
                    pe.wait_ge(sx[g], 64)
                    for q in range(NQ):
                        t = g * NQ + q
                        if t >= 4:
                            pe.wait_ge(s_ev1, t - 3)
                        last = None
                        for xj in range(XQ):
                            x = XQ * q + xj
                            for h in range(2):
                                lhsT = bass.AP(
                                    tensor=x1s,
                                    offset=h * H * W + YT * g * W + x,
                                    ap=[[2 * H * W, 128], [W, YT]],
                                )
                                rhs = bass.AP(
                                    tensor=x2s,
                                    offset=h * X2R * X2C + YT * g * X2C + x,
                                    ap=[
                                        [2 * X2R * X2C, 128],
                                        [X2C, WU],
                                        [1, WV],
                                    ],
                                )
                                last = nc.tensor.matmul(
                                    pg[t % 4].ap()[
                                        YT * xj : YT * xj + YT, 0:NBHD
                                    ],
                                    lhsT,
                                    rhs,
                                    start=(h == 0),
                                    stop=(h == 1),
                                    tile_position=(0, YT * xj),
                                )
                        last.then_inc(s_mm, 1)
                        if g == 2:
                            x_hi = XQ * q + XQ - 1
                            k = x_hi - LAG
                            for kk in range(max(0, k - XQ + 1), max(0, k + 1)):
                                transpose_band(pe, kk)
                for k in range(max(0, W - LAG), W):
                    transpose_band(pe, k)

    return nc


def kernel(x1, x2, trace=False):
    n = x1.shape[0]
    nc = build()
    in_maps = [
        {
            "x1": np.ascontiguousarray(x1[i], dtype=np.float32),
            "x2": np.ascontiguousarray(x2[i], dtype=np.float32),
        }
        for i in range(n)
    ]
    res = run_bass_kernel_spmd(nc, in_maps, list(range(n)), trace=trace)
    outv = np.stack([r["out"] for r in res.results], axis=0)
    if trace:
        kernel.last_exec_time_ns = res.exec_time_ns
        kernel.last_trace = res.instructions_and_trace
    return outv
